# revision 10
# baseline (speedup 1.0000x reference)
"""Trainium2 Bass kernel for CustomMinkowskiLayerNorm.

Math (matches the jax reference):
    counts[b]  = #points with batch_indices == b           (clamped >= 1)
    mean[b,c]  = sum_{i in b} x[i,c] / counts[b]
    var[b,c]   = sum_{i in b} (x[i,c]-mean)^2 / counts[b]  (= E[x^2]-mean^2)
    out[i,c]   = (x[i,c]-mean[b_i,c]) / sqrt(var[b_i,c]+eps) * gamma[c] + beta[c]

Sharding: batch_indices is sorted and BATCH == n_cores == 8, so each core owns
exactly one batch segment -> all segment reductions are core-local, no
collectives. The host splits at segment boundaries (searchsorted), transposes
each segment to channel-major layout and zero-pads to a fixed shape:

    xt[p, f], p in [0,128): partition p < 64  = channel p,  points [0, F_HALF)
                            partition p >= 64 = channel p-64, points [F_HALF, 2*F_HALF)

Channel-major layout makes the per-channel segment reduction a free-dim
reduction (bn_stats) and the normalization a single per-partition
tensor_scalar (x*s + t) that runs in the DVE 2x fp32 perf mode.

Device program (per core, identical SPMD):
  pass 1: DMA tiles of [128, <=2048]; one bn_stats per 512-chunk into a stats
          buffer; the first NCACHE tiles stay resident in SBUF. Pass-2
          re-reads for the non-resident tiles are issued right after on the
          SWDGE ring so they fill DMA gaps while DVE catches up.
  stats:  bn_aggr (split: everything-but-last-tile early, last tile late) ->
          (mean, var) -> raw (sum, sumsq); fold partitions p/p+64 and
          broadcast to both halves with one TensorE matmul against a 0/1
          fold matrix; apply 1/count; rstd = 1/sqrt(var+eps) refined with
          2 Newton iterations (ACT sqrt table is low-precision);
          s = gamma*rstd, t = beta - mean*s.
  pass 2: x_tile = x_tile * s + t in place (tensor_scalar); DMA back. Only
          non-resident tiles are re-read from HBM.
"""

import os
import sys

for _p in ("/opt/trn_rl_repo", "/root/.axon_site/_ro/trn_rl_repo"):
    if os.path.isdir(_p) and _p not in sys.path:
        sys.path.append(_p)

from contextlib import ExitStack

import numpy as np

import concourse.bacc as bacc
import concourse.tile as tile
from concourse import mybir
from concourse._compat import with_exitstack
from concourse.bass_utils import run_bass_kernel_spmd

F32 = mybir.dt.float32

N = 1_000_000
C = 64
BATCH = 8
EPS = 1e-5

P = 128            # SBUF partitions
F_TILE = 2048      # free elems per full tile -> [128, 2048] f32 = 1 MiB DMA
BN_F = 512         # bn_stats free-dim max (and f_half granularity)
NCACHE_MAX = 19    # tiles kept resident in SBUF between passes
LOAD_BUFS = 3      # rotating pass-1 load slots
P2_BUFS = 3        # rotating pass-2 re-read slots

_mult = mybir.AluOpType.mult
_add = mybir.AluOpType.add


def _tile_sizes(f_half: int):
    sizes = [F_TILE] * (f_half // F_TILE)
    if f_half % F_TILE:
        sizes.append(f_half % F_TILE)
    return sizes


def _make_body(f_half: int):
    sizes = _tile_sizes(f_half)
    nt = len(sizes)
    offs = [sum(sizes[:i]) for i in range(nt)]
    ngroups = [s // BN_F for s in sizes]
    goffs = [sum(ngroups[:i]) for i in range(nt)]
    gtot = sum(ngroups)
    ncache = min(NCACHE_MAX, nt)

    @with_exitstack
    def _body(ctx: ExitStack, tc: tile.TileContext,
              out_ap, xt_ap, invn_ap, gcol_ap, bcol_ap, foldm_ap):
        nc = tc.nc

        cache = ctx.enter_context(tc.tile_pool(name="cache", bufs=ncache))
        lpool = ctx.enter_context(tc.tile_pool(name="lpool", bufs=LOAD_BUFS))
        p2pool = ctx.enter_context(tc.tile_pool(name="p2pool", bufs=P2_BUFS))
        small = ctx.enter_context(tc.tile_pool(name="small", bufs=1))
        psum = ctx.enter_context(tc.tile_pool(name="psum", bufs=1, space="PSUM"))

        stats = small.tile([P, gtot, 6], F32, tag="stats")

        # ---- pass 1: stream all tiles, bn_stats each 512-chunk ----
        cached = {}
        p1tiles = {}
        for t in range(nt):
            sl = slice(offs[t], offs[t] + sizes[t])
            if t < ncache:
                xt = cache.tile([P, sizes[t]], F32, tag="c")
                cached[t] = xt
            else:
                xt = lpool.tile([P, sizes[t]], F32, tag="l")
            nc.sync.dma_start(out=xt, in_=xt_ap[:, sl])
            p1tiles[t] = xt

        # Small inputs on the SWDGE ring, issued after the big loads so they
        # don't head-block the sync HWDGE FIFO at kernel start.
        invn_sb = small.tile([P, 1], F32, tag="invn")
        gcol_sb = small.tile([P, 1], F32, tag="gcol")
        bcol_sb = small.tile([P, 1], F32, tag="bcol")
        foldm_sb = small.tile([P, P], F32, tag="foldm")
        nc.gpsimd.dma_start(out=invn_sb, in_=invn_ap)
        nc.gpsimd.dma_start(out=gcol_sb, in_=gcol_ap)
        nc.gpsimd.dma_start(out=bcol_sb, in_=bcol_ap)
        nc.gpsimd.dma_start(out=foldm_sb, in_=foldm_ap)

        # Pre-load the ACT sqrt table set while DMA streams pass 1, so the
        # stats chain later doesn't stall on ACT_TABLE_LOAD.
        warm = small.tile([P, 1], F32, tag="warm")
        nc.vector.memset(warm, 1.0)
        nc.scalar.activation(out=warm, in_=warm,
                             func=mybir.ActivationFunctionType.Sqrt)

        # bn_stats in pass-1 program order
        for t in range(nt):
            xt = p1tiles[t]
            for j in range(ngroups[t]):
                nc.vector.bn_stats(
                    out=stats[:, goffs[t] + j, :],
                    in_=xt[:, j * BN_F : (j + 1) * BN_F],
                )

        # Issue pass-2 re-read DMAs now, on the SWDGE ring: the sync HWDGE
        # ring is FIFO and its tail loads are DVE-gated, so these would
        # otherwise queue behind them instead of filling idle DMA bandwidth.
        p2tiles = {}
        for t in range(ncache, nt):
            sl = slice(offs[t], offs[t] + sizes[t])
            xt = p2pool.tile([P, sizes[t]], F32, tag="p2")
            nc.gpsimd.dma_start(out=xt, in_=xt_ap[:, sl])
            p2tiles[t] = xt

        # ---- aggregate stats ----
        # Split so only the last tile's groups aggregate on the critical path.
        ga = gtot - ngroups[nt - 1]
        na, nb_ = ga * BN_F, ngroups[nt - 1] * BN_F
        mva = small.tile([P, 2], F32, tag="mva")
        mvb = small.tile([P, 2], F32, tag="mvb")
        nc.vector.bn_aggr(out=mva, in_=stats[:, :ga, :])
        nc.vector.bn_aggr(out=mvb, in_=stats[:, ga:, :])

        def raw_sums(dst, mv, n):
            # dst[:,0] = mean*n ; dst[:,1] = (var+mean^2)*n
            m2 = small.tile([P, 1], F32, tag="m2tmp")
            nc.vector.tensor_mul(out=m2, in0=mv[:, 0:1], in1=mv[:, 0:1])
            nc.vector.tensor_add(out=m2, in0=m2, in1=mv[:, 1:2])
            nc.vector.tensor_scalar_mul(out=dst[:, 0:1], in0=mv[:, 0:1],
                                        scalar1=float(n))
            nc.vector.tensor_scalar_mul(out=dst[:, 1:2], in0=m2,
                                        scalar1=float(n))

        sums_a = small.tile([P, 2], F32, tag="sums_a")
        sums_b = small.tile([P, 2], F32, tag="sums_b")
        raw_sums(sums_a, mva, na)
        raw_sums(sums_b, mvb, nb_)
        sums = small.tile([P, 2], F32, tag="sums")
        nc.vector.tensor_add(out=sums, in0=sums_a, in1=sums_b)

        # ---- fold halves + broadcast: tot[p] = sums[p%64] + sums[p%64+64] ----
        ptot = psum.tile([P, 2], F32, tag="pt")
        nc.tensor.matmul(out=ptot, lhsT=foldm_sb, rhs=sums,
                         start=True, stop=True)
        tot = small.tile([P, 2], F32, tag="tot")
        nc.vector.tensor_copy(out=tot, in_=ptot)

        # ---- per-channel coefficients ----
        mm = small.tile([P, 2], F32, tag="mm")      # (mean, E[x^2])
        nc.vector.tensor_scalar_mul(out=mm, in0=tot, scalar1=invn_sb[:, 0:1])
        var = small.tile([P, 1], F32, tag="var")
        nc.vector.tensor_mul(out=var, in0=mm[:, 0:1], in1=mm[:, 0:1])
        nc.vector.tensor_sub(out=var, in0=mm[:, 1:2], in1=var)
        # v = max(var, 0) + eps
        v = small.tile([P, 1], F32, tag="v")
        nc.vector.tensor_scalar(out=v, in0=var, scalar1=0.0, scalar2=EPS,
                                op0=mybir.AluOpType.max, op1=_add)
        # r ~= 1/sqrt(v): ACT sqrt + reciprocal, then 2 Newton steps
        r = small.tile([P, 1], F32, tag="r")
        nc.scalar.activation(out=r, in_=v,
                             func=mybir.ActivationFunctionType.Sqrt)
        nc.vector.reciprocal(out=r, in_=r)
        a = small.tile([P, 1], F32, tag="a")
        for _ in range(2):
            nc.vector.tensor_mul(out=a, in0=r, in1=r)
            nc.vector.tensor_mul(out=a, in0=a, in1=v)
            nc.vector.tensor_scalar(out=a, in0=a, scalar1=-0.5, scalar2=1.5,
                                    op0=_mult, op1=_add)
            nc.vector.tensor_mul(out=r, in0=r, in1=a)
        s_col = small.tile([P, 1], F32, tag="s_col")
        nc.vector.tensor_mul(out=s_col, in0=r, in1=gcol_sb)
        t_col = small.tile([P, 1], F32, tag="t_col")
        nc.vector.tensor_mul(out=t_col, in0=mm[:, 0:1], in1=s_col)
        nc.vector.tensor_sub(out=t_col, in0=bcol_sb, in1=t_col)

        # ---- pass 2: x = x*s + t, store ----
        # interleave non-resident tiles among resident ones so their re-read
        # slots recycle while stores stream
        cu, uu = list(range(ncache)), list(range(ncache, nt))
        order = []
        while cu or uu:
            if uu:
                order.append(uu.pop(0))
            order.extend(cu[:2])
            del cu[:2]
        for t in order:
            sl = slice(offs[t], offs[t] + sizes[t])
            xt = cached[t] if t < ncache else p2tiles[t]
            nc.vector.tensor_scalar(out=xt, in0=xt, scalar1=s_col[:, 0:1],
                                    scalar2=t_col[:, 0:1], op0=_mult, op1=_add)
            nc.scalar.dma_start(out=out_ap[:, sl], in_=xt)

    return _body


_NC_CACHE = {}


def _build_program(f_half: int):
    if f_half in _NC_CACHE:
        return _NC_CACHE[f_half]
    nc = bacc.Bacc("TRN2", target_bir_lowering=False, debug=False,
                   num_devices=BATCH)
    xt = nc.dram_tensor("xt", [P, f_half], F32, kind="ExternalInput").ap()
    invn = nc.dram_tensor("invn", [P, 1], F32, kind="ExternalInput").ap()
    gcol = nc.dram_tensor("gcol", [P, 1], F32, kind="ExternalInput").ap()
    bcol = nc.dram_tensor("bcol", [P, 1], F32, kind="ExternalInput").ap()
    foldm = nc.dram_tensor("foldm", [P, P], F32, kind="ExternalInput").ap()
    out = nc.dram_tensor("out", [P, f_half], F32, kind="ExternalOutput").ap()
    with tile.TileContext(nc) as tc:
        _make_body(f_half)(tc, out, xt, invn, gcol, bcol, foldm)
    nc.compile()
    _NC_CACHE[f_half] = nc
    return nc


def _prepare(features, batch_indices, gamma, beta):
    features = np.asarray(features, dtype=np.float32)
    batch_indices = np.asarray(batch_indices, dtype=np.int32)
    gamma = np.asarray(gamma, dtype=np.float32)
    beta = np.asarray(beta, dtype=np.float32)

    bounds = np.searchsorted(batch_indices, np.arange(BATCH + 1), side="left")
    cnts = np.diff(bounds)
    # fixed SPMD shape: half-row length, padded to a multiple of BN_F
    f_half = max(int(-(-int(cnts.max()) // 2 // BN_F) * BN_F), BN_F)

    gcol = np.concatenate([gamma, gamma]).reshape(P, 1).astype(np.float32)
    bcol = np.concatenate([beta, beta]).reshape(P, 1).astype(np.float32)
    k = np.arange(P)
    foldm = (k[:, None] % C == k[None, :] % C).astype(np.float32)

    in_maps = []
    for b in range(BATCH):
        s, e = int(bounds[b]), int(bounds[b + 1])
        cnt = e - s
        xt = np.zeros((P, f_half), dtype=np.float32)
        n1 = min(cnt, f_half)
        if n1 > 0:
            xt[0:C, :n1] = features[s : s + n1].T
        if cnt > f_half:
            xt[C:P, : cnt - f_half] = features[s + f_half : e].T
        in_maps.append({
            "xt": xt,
            "invn": np.full((P, 1), 1.0 / max(cnt, 1), dtype=np.float32),
            "gcol": gcol,
            "bcol": bcol,
            "foldm": foldm,
        })
    return in_maps, bounds, f_half


def _assemble(results, bounds, f_half):
    out = np.empty((N, C), dtype=np.float32)
    for b in range(BATCH):
        s, e = int(bounds[b]), int(bounds[b + 1])
        cnt = e - s
        if cnt == 0:
            continue
        ot = results[b]["out"]
        n1 = min(cnt, f_half)
        out[s : s + n1] = ot[0:C, :n1].T
        if cnt > f_half:
            out[s + f_half : e] = ot[C:P, : cnt - f_half].T
    return out


def run_with_results(features, batch_indices, gamma, beta, **run_kwargs):
    in_maps, bounds, f_half = _prepare(features, batch_indices, gamma, beta)
    nc = _build_program(f_half)
    res = run_bass_kernel_spmd(nc, in_maps, core_ids=list(range(BATCH)),
                               **run_kwargs)
    return _assemble(res.results, bounds, f_half), res


def kernel(features, batch_indices, gamma, beta):
    out, _ = run_with_results(features, batch_indices, gamma, beta)
    return out


# revision 11
# speedup vs baseline: 1.0489x; 1.0489x over previous
"""Trainium2 Bass kernel for CustomMinkowskiLayerNorm.

Math (matches the jax reference):
    counts[b]  = #points with batch_indices == b           (clamped >= 1)
    mean[b,c]  = sum_{i in b} x[i,c] / counts[b]
    var[b,c]   = sum_{i in b} (x[i,c]-mean)^2 / counts[b]  (= E[x^2]-mean^2)
    out[i,c]   = (x[i,c]-mean[b_i,c]) / sqrt(var[b_i,c]+eps) * gamma[c] + beta[c]

Sharding: batch_indices is sorted and BATCH == n_cores == 8, so each core owns
exactly one batch segment -> all segment reductions are core-local, no
collectives. The host splits at segment boundaries (searchsorted), transposes
each segment to channel-major layout and zero-pads to a fixed shape:

    xt[p, f], p in [0,128): partition p < 64  = channel p,  points [0, F_HALF)
                            partition p >= 64 = channel p-64, points [F_HALF, 2*F_HALF)

Channel-major layout makes the per-channel segment reduction a free-dim
reduction (bn_stats) and the normalization a single per-partition
tensor_scalar (x*s + t) that runs in the DVE 2x fp32 perf mode.

Device program (per core, identical SPMD):
  pass 1: DMA tiles of [128, <=2048]; one bn_stats per 512-chunk into a stats
          buffer; the first NCACHE tiles stay resident in SBUF. Pass-2
          re-reads for the non-resident tiles are issued right after on the
          SWDGE ring so they fill DMA gaps while DVE catches up.
  stats:  bn_aggr (split: everything-but-last-tile early, last tile late) ->
          (mean, var) -> raw (sum, sumsq); fold partitions p/p+64 and
          broadcast to both halves with one TensorE matmul against a 0/1
          fold matrix; apply 1/count; rstd = 1/sqrt(var+eps) refined with
          2 Newton iterations (ACT sqrt table is low-precision);
          s = gamma*rstd, t = beta - mean*s.
  pass 2: x_tile = x_tile * s + t in place (tensor_scalar); DMA back. Only
          non-resident tiles are re-read from HBM.
"""

import os
import sys

for _p in ("/opt/trn_rl_repo", "/root/.axon_site/_ro/trn_rl_repo"):
    if os.path.isdir(_p) and _p not in sys.path:
        sys.path.append(_p)

from contextlib import ExitStack

import numpy as np

import concourse.bacc as bacc
import concourse.tile as tile
from concourse import mybir
from concourse._compat import with_exitstack
from concourse.bass_utils import run_bass_kernel_spmd

F32 = mybir.dt.float32

N = 1_000_000
C = 64
BATCH = 8
EPS = 1e-5

P = 128            # SBUF partitions
F_TILE = 2048      # free elems per full tile -> [128, 2048] f32 = 1 MiB DMA
BN_F = 512         # bn_stats free-dim max (and f_half granularity)
NCACHE_MAX = 19    # tiles kept resident in SBUF between passes
LOAD_BUFS = 3      # rotating pass-1 load slots
P2_BUFS = 3        # rotating pass-2 re-read slots

_mult = mybir.AluOpType.mult
_add = mybir.AluOpType.add


def _tile_sizes(f_half: int):
    sizes = [F_TILE] * (f_half // F_TILE)
    if f_half % F_TILE:
        sizes.append(f_half % F_TILE)
    return sizes


def _make_body(f_half: int):
    sizes = _tile_sizes(f_half)
    nt = len(sizes)
    offs = [sum(sizes[:i]) for i in range(nt)]
    ngroups = [s // BN_F for s in sizes]
    goffs = [sum(ngroups[:i]) for i in range(nt)]
    gtot = sum(ngroups)
    ncache = min(NCACHE_MAX, nt)

    @with_exitstack
    def _body(ctx: ExitStack, tc: tile.TileContext,
              out_ap, xt_ap, invn_ap, gcol_ap, bcol_ap, foldm_ap):
        nc = tc.nc

        cache = ctx.enter_context(tc.tile_pool(name="cache", bufs=ncache))
        lpool = ctx.enter_context(tc.tile_pool(name="lpool", bufs=LOAD_BUFS))
        p2pool = ctx.enter_context(tc.tile_pool(name="p2pool", bufs=P2_BUFS))
        small = ctx.enter_context(tc.tile_pool(name="small", bufs=1))
        psum = ctx.enter_context(tc.tile_pool(name="psum", bufs=1, space="PSUM"))

        stats = small.tile([P, gtot, 6], F32, tag="stats")

        # Small inputs on the scalar HWDGE ring: its first stores come much
        # later, so these don't head-block anything.
        invn_sb = small.tile([P, 1], F32, tag="invn")
        gcol_sb = small.tile([P, 1], F32, tag="gcol")
        bcol_sb = small.tile([P, 1], F32, tag="bcol")
        foldm_sb = small.tile([P, P], F32, tag="foldm")
        nc.scalar.dma_start(out=invn_sb, in_=invn_ap)
        nc.scalar.dma_start(out=gcol_sb, in_=gcol_ap)
        nc.scalar.dma_start(out=bcol_sb, in_=bcol_ap)
        nc.scalar.dma_start(out=foldm_sb, in_=foldm_ap)

        # Pre-load the ACT sqrt table set while DMA streams pass 1, so the
        # stats chain later doesn't stall on ACT_TABLE_LOAD.
        warm = small.tile([P, 1], F32, tag="warm")
        nc.vector.memset(warm, 1.0)
        nc.scalar.activation(out=warm, in_=warm,
                             func=mybir.ActivationFunctionType.Sqrt)

        # ---- pass 1: stream all tiles, bn_stats each 512-chunk ----
        cached = {}
        for t in range(nt):
            sl = slice(offs[t], offs[t] + sizes[t])
            if t < ncache:
                xt = cache.tile([P, sizes[t]], F32, tag="c")
                cached[t] = xt
            else:
                xt = lpool.tile([P, sizes[t]], F32, tag="l")
            nc.sync.dma_start(out=xt, in_=xt_ap[:, sl])
            for j in range(ngroups[t]):
                nc.vector.bn_stats(
                    out=stats[:, goffs[t] + j, :],
                    in_=xt[:, j * BN_F : (j + 1) * BN_F],
                )

        # Pass-2 re-read DMAs, queued on the sync HWDGE ring behind the
        # DVE-gated pass-1 tail loads: ring FIFO delays them exactly until
        # pass 1 stops needing the bandwidth, then they stream through the
        # stats chain and into pass 2.
        p2tiles = {}
        for t in range(ncache, nt):
            sl = slice(offs[t], offs[t] + sizes[t])
            xt = p2pool.tile([P, sizes[t]], F32, tag="p2")
            nc.sync.dma_start(out=xt, in_=xt_ap[:, sl])
            p2tiles[t] = xt

        # ---- aggregate stats ----
        # Split so only the last tile's groups aggregate on the critical path.
        ga = gtot - ngroups[nt - 1]
        na, nb_ = ga * BN_F, ngroups[nt - 1] * BN_F
        mva = small.tile([P, 2], F32, tag="mva")
        mvb = small.tile([P, 2], F32, tag="mvb")
        nc.vector.bn_aggr(out=mva, in_=stats[:, :ga, :])
        nc.vector.bn_aggr(out=mvb, in_=stats[:, ga:, :])

        def raw_sums(dst, mv, n):
            # dst[:,0] = mean*n ; dst[:,1] = (var+mean^2)*n
            m2 = small.tile([P, 1], F32, tag="m2tmp")
            nc.vector.tensor_mul(out=m2, in0=mv[:, 0:1], in1=mv[:, 0:1])
            nc.vector.tensor_add(out=m2, in0=m2, in1=mv[:, 1:2])
            nc.vector.tensor_scalar_mul(out=dst[:, 0:1], in0=mv[:, 0:1],
                                        scalar1=float(n))
            nc.vector.tensor_scalar_mul(out=dst[:, 1:2], in0=m2,
                                        scalar1=float(n))

        sums_a = small.tile([P, 2], F32, tag="sums_a")
        sums_b = small.tile([P, 2], F32, tag="sums_b")
        raw_sums(sums_a, mva, na)
        raw_sums(sums_b, mvb, nb_)
        sums = small.tile([P, 2], F32, tag="sums")
        nc.vector.tensor_add(out=sums, in0=sums_a, in1=sums_b)

        # ---- fold halves + broadcast: tot[p] = sums[p%64] + sums[p%64+64] ----
        ptot = psum.tile([P, 2], F32, tag="pt")
        nc.tensor.matmul(out=ptot, lhsT=foldm_sb, rhs=sums,
                         start=True, stop=True)
        tot = small.tile([P, 2], F32, tag="tot")
        nc.vector.tensor_copy(out=tot, in_=ptot)

        # ---- per-channel coefficients ----
        mm = small.tile([P, 2], F32, tag="mm")      # (mean, E[x^2])
        nc.vector.tensor_scalar_mul(out=mm, in0=tot, scalar1=invn_sb[:, 0:1])
        var = small.tile([P, 1], F32, tag="var")
        nc.vector.tensor_mul(out=var, in0=mm[:, 0:1], in1=mm[:, 0:1])
        nc.vector.tensor_sub(out=var, in0=mm[:, 1:2], in1=var)
        # v = max(var, 0) + eps
        v = small.tile([P, 1], F32, tag="v")
        nc.vector.tensor_scalar(out=v, in0=var, scalar1=0.0, scalar2=EPS,
                                op0=mybir.AluOpType.max, op1=_add)
        # r ~= 1/sqrt(v): ACT sqrt + reciprocal, then 2 Newton steps
        r = small.tile([P, 1], F32, tag="r")
        nc.scalar.activation(out=r, in_=v,
                             func=mybir.ActivationFunctionType.Sqrt)
        nc.vector.reciprocal(out=r, in_=r)
        a = small.tile([P, 1], F32, tag="a")
        for _ in range(2):
            nc.vector.tensor_mul(out=a, in0=r, in1=r)
            nc.vector.tensor_mul(out=a, in0=a, in1=v)
            nc.vector.tensor_scalar(out=a, in0=a, scalar1=-0.5, scalar2=1.5,
                                    op0=_mult, op1=_add)
            nc.vector.tensor_mul(out=r, in0=r, in1=a)
        s_col = small.tile([P, 1], F32, tag="s_col")
        nc.vector.tensor_mul(out=s_col, in0=r, in1=gcol_sb)
        t_col = small.tile([P, 1], F32, tag="t_col")
        nc.vector.tensor_mul(out=t_col, in0=mm[:, 0:1], in1=s_col)
        nc.vector.tensor_sub(out=t_col, in0=bcol_sb, in1=t_col)

        # ---- pass 2: x = x*s + t, store ----
        # interleave non-resident tiles among resident ones so their re-read
        # slots recycle while stores stream
        cu, uu = list(range(ncache)), list(range(ncache, nt))
        order = []
        while cu or uu:
            if uu:
                order.append(uu.pop(0))
            order.extend(cu[:2])
            del cu[:2]
        for t in order:
            sl = slice(offs[t], offs[t] + sizes[t])
            xt = cached[t] if t < ncache else p2tiles[t]
            nc.vector.tensor_scalar(out=xt, in0=xt, scalar1=s_col[:, 0:1],
                                    scalar2=t_col[:, 0:1], op0=_mult, op1=_add)
            nc.scalar.dma_start(out=out_ap[:, sl], in_=xt)

    return _body


_NC_CACHE = {}


def _build_program(f_half: int):
    if f_half in _NC_CACHE:
        return _NC_CACHE[f_half]
    nc = bacc.Bacc("TRN2", target_bir_lowering=False, debug=False,
                   num_devices=BATCH)
    xt = nc.dram_tensor("xt", [P, f_half], F32, kind="ExternalInput").ap()
    invn = nc.dram_tensor("invn", [P, 1], F32, kind="ExternalInput").ap()
    gcol = nc.dram_tensor("gcol", [P, 1], F32, kind="ExternalInput").ap()
    bcol = nc.dram_tensor("bcol", [P, 1], F32, kind="ExternalInput").ap()
    foldm = nc.dram_tensor("foldm", [P, P], F32, kind="ExternalInput").ap()
    out = nc.dram_tensor("out", [P, f_half], F32, kind="ExternalOutput").ap()
    with tile.TileContext(nc) as tc:
        _make_body(f_half)(tc, out, xt, invn, gcol, bcol, foldm)
    nc.compile()
    _NC_CACHE[f_half] = nc
    return nc


def _prepare(features, batch_indices, gamma, beta):
    features = np.asarray(features, dtype=np.float32)
    batch_indices = np.asarray(batch_indices, dtype=np.int32)
    gamma = np.asarray(gamma, dtype=np.float32)
    beta = np.asarray(beta, dtype=np.float32)

    bounds = np.searchsorted(batch_indices, np.arange(BATCH + 1), side="left")
    cnts = np.diff(bounds)
    # fixed SPMD shape: half-row length, padded to a multiple of BN_F
    f_half = max(int(-(-int(cnts.max()) // 2 // BN_F) * BN_F), BN_F)

    gcol = np.concatenate([gamma, gamma]).reshape(P, 1).astype(np.float32)
    bcol = np.concatenate([beta, beta]).reshape(P, 1).astype(np.float32)
    k = np.arange(P)
    foldm = (k[:, None] % C == k[None, :] % C).astype(np.float32)

    in_maps = []
    for b in range(BATCH):
        s, e = int(bounds[b]), int(bounds[b + 1])
        cnt = e - s
        xt = np.zeros((P, f_half), dtype=np.float32)
        n1 = min(cnt, f_half)
        if n1 > 0:
            xt[0:C, :n1] = features[s : s + n1].T
        if cnt > f_half:
            xt[C:P, : cnt - f_half] = features[s + f_half : e].T
        in_maps.append({
            "xt": xt,
            "invn": np.full((P, 1), 1.0 / max(cnt, 1), dtype=np.float32),
            "gcol": gcol,
            "bcol": bcol,
            "foldm": foldm,
        })
    return in_maps, bounds, f_half


def _assemble(results, bounds, f_half):
    out = np.empty((N, C), dtype=np.float32)
    for b in range(BATCH):
        s, e = int(bounds[b]), int(bounds[b + 1])
        cnt = e - s
        if cnt == 0:
            continue
        ot = results[b]["out"]
        n1 = min(cnt, f_half)
        out[s : s + n1] = ot[0:C, :n1].T
        if cnt > f_half:
            out[s + f_half : e] = ot[C:P, : cnt - f_half].T
    return out


def run_with_results(features, batch_indices, gamma, beta, **run_kwargs):
    in_maps, bounds, f_half = _prepare(features, batch_indices, gamma, beta)
    nc = _build_program(f_half)
    res = run_bass_kernel_spmd(nc, in_maps, core_ids=list(range(BATCH)),
                               **run_kwargs)
    return _assemble(res.results, bounds, f_half), res


def kernel(features, batch_indices, gamma, beta):
    out, _ = run_with_results(features, batch_indices, gamma, beta)
    return out
